# revision 43
# baseline (speedup 1.0000x reference)
"""Banked linear (MoE routing) kernel for 8 Trainium2 NeuronCores.

Problem: out[b,s,k,:] = tensor[b,s,k,:] @ weight[sel[b,s,k]].T + bias[sel[b,s,k]]
Shapes: tensor (2,256,2,512), sel (2,256,2) int, weight (16,512,512), bias (16,512).

Strategy (expert-parallel, host-routed dispatch):
  * Flatten to 1024 token-slots; group them by selected bank on the host
    (the "all-to-all" of the sharding hint, done during input sharding).
  * 16 banks -> 8 cores, 2 banks per core. Each core reads only its own
    2 banks' weights (16 MiB of weights read exactly once across the chip).
  * Per bank: tokens padded to capacity C, x transposed host-side so the
    device does   psum[C,512] = sum_k xT[k*128:+128, :C].T @ WT[k*128:+128, :512]
  * Outputs scattered back to (B,S,K,OUT) positions on the host; the bias
    gather/add rides along with the scatter (O(out) host work).

General-case fallback: if a bank attracts more than 128 token-slots the
bank is split into several jobs of <=128 tokens (weights re-read per job).
"""

import numpy as np

import concourse.bacc as bacc
import concourse.bass as bass
import concourse.mybir as mybir
import concourse.tile as tile
from concourse.bass_utils import run_bass_kernel_spmd

B, S, K = 2, 256, 2
IN, OUT, NB = 512, 512, 16
N_CORES = 8
P = 128  # partition dim / contraction tile

_MODULES: dict = {}  # (jobs_per_core, capacity) -> compiled bass module
LAST_RESULTS = None  # BassKernelResults of the most recent run (for test.py)


MM_DT = mybir.dt.float32  # full-precision matmul (f32r would be ~13% faster
                          # at ~1.3e-4 rel err; fp32 keeps 1.3e-7)
WARMUP_MMS = 14  # full-width dummy matmuls warm HAM to 2.4 GHz (-0.9 us)
DMA_SCRATCH = 16384  # Bass dynamic_dma_scratch_size
SPLIT_COPY = False  # gpsimd cannot read PSUM; only DVE does this well
W_SPLIT = "ksync"  # weight DMA granularity: "k" | "expert" | "half" | "ksync"
OUT_RING = "sync"   # ring for output DMAs: "sync" | "scalar"
NO_PARTITION_ID = True  # skip partition-id preamble machinery


def _build_module(jpc: int, cap: int) -> bass.Bass:
    f32 = mybir.dt.float32
    mdt = MM_DT
    kt = IN // P
    nc = bacc.Bacc(None, target_bir_lowering=False, debug=False,
                   enable_partition_id=not NO_PARTITION_ID,
                   dynamic_dma_scratch_size=DMA_SCRATCH)
    # x pre-swizzled host-side to [p, j, k, t] so this DMA is contiguous
    xt = nc.dram_tensor("xt", (P, jpc, kt, cap), mdt, kind="ExternalInput")
    # weights pre-swizzled host-side to [j, p, k, n]: contiguous per job
    wt = nc.dram_tensor("wt", (jpc, P, kt, OUT), mdt, kind="ExternalInput")
    out = nc.dram_tensor("out", (jpc, cap, OUT), f32, kind="ExternalOutput")
    dbg = (nc.dram_tensor("dbg", (1, 1), f32, kind="ExternalOutput")
           if WARMUP_MMS else None)

    with tile.TileContext(nc) as tc:
        with (
            tc.tile_pool(name="wp", bufs=jpc) as wp,
            tc.tile_pool(name="xp", bufs=1) as xp,
            tc.tile_pool(name="op", bufs=2) as op,
            tc.tile_pool(name="warm", bufs=1) as wmp,
            tc.tile_pool(name="ps", bufs=2, space="PSUM") as pp,
            tc.tile_pool(name="pswarm", bufs=1, space="PSUM") as ppw,
        ):
            # all jobs' x in one contiguous DMA, first on the scalar ring
            xsb = xp.tile([P, jpc, kt, cap], mdt)
            nc.scalar.dma_start(xsb[:], xt[:])
            # per-job weights; DMA granularity/ring assignment per W_SPLIT
            wsb = []
            ring_i = 0
            for j in range(jpc):
                w = wp.tile([P, kt, OUT], mdt)
                if W_SPLIT == "expert":
                    ring = nc.sync if j % 2 == 0 else nc.scalar
                    ring.dma_start(w[:], wt[j])
                elif W_SPLIT == "half":
                    h = kt // 2
                    nc.sync.dma_start(w[:, :h, :], wt[j, :, :h, :])
                    nc.scalar.dma_start(w[:, h:, :], wt[j, :, h:, :])
                elif W_SPLIT == "ksync":
                    # all weight tiles on the sync ring, consumption order
                    for k in range(kt):
                        nc.sync.dma_start(w[:, k, :], wt[j, :, k, :])
                else:  # "k": one DMA per k-tile, alternating rings
                    for k in range(kt):
                        ring = nc.sync if ring_i % 2 == 0 else nc.scalar
                        ring.dma_start(w[:, k, :], wt[j, :, k, :])
                        ring_i += 1
                wsb.append(w)
            # PE warm-up spin: full-width dummy bf16 matmuls while the
            # weight DMAs are in flight (HAM un-throttles after ~3.4 us of
            # genuine array activity; a 1-wide matmul does not count)
            if WARMUP_MMS:
                bf16 = mybir.dt.bfloat16
                wz = wmp.tile([P, P + OUT], bf16)
                nc.vector.memset(wz[:], 0.0)
                wps = ppw.tile([P, OUT], f32)
                for _ in range(WARMUP_MMS):
                    nc.tensor.matmul(wps[:], wz[:, :P], wz[:, P:],
                                     start=True, stop=True)
                # tiny consumer so the spin isn't dead-code-eliminated
                wdbg = wmp.tile([1, 1], f32)
                nc.vector.tensor_copy(wdbg[:], wps[:1, :1])
                nc.scalar.dma_start(dbg[:], wdbg[:])
            for j in range(jpc):
                if SPLIT_COPY:
                    # two 256-col PSUM groups: first half's copy+store
                    # overlaps the second half's matmuls, shrinking the
                    # post-PE tail
                    h = OUT // 2
                    for half in range(2):
                        psum = pp.tile([cap, h], f32, tag=f"ps{half}")
                        lo = half * h
                        for k in range(kt):
                            nc.tensor.matmul(
                                psum[:], xsb[:, j, k, :],
                                wsb[j][:, k, lo:lo + h],
                                start=(k == 0), stop=(k == kt - 1))
                        osb = op.tile([cap, h], f32, tag=f"os{half}")
                        nc.vector.tensor_copy(osb[:], psum[:])
                        ring = nc.scalar if half == 0 else nc.sync
                        ring.dma_start(out[j, :, lo:lo + h], osb[:])
                else:
                    psum = pp.tile([cap, OUT], f32)
                    for k in range(kt):
                        nc.tensor.matmul(psum[:], xsb[:, j, k, :],
                                         wsb[j][:, k, :],
                                         start=(k == 0), stop=(k == kt - 1))
                    # bias is added host-side on scatter
                    osb = op.tile([cap, OUT], f32)
                    nc.vector.tensor_copy(osb[:], psum[:])
                    out_ring = nc.sync if OUT_RING == "sync" else nc.scalar
                    out_ring.dma_start(out[j], osb[:])
    nc.compile()
    return nc


def _get_module(jpc: int, cap: int) -> bass.Bass:
    key = (jpc, cap)
    if key not in _MODULES:
        _MODULES[key] = _build_module(jpc, cap)
    return _MODULES[key]


def kernel(tensor, bank_selections, weight, bias):
    global LAST_RESULTS
    tensor = np.asarray(tensor, dtype=np.float32)
    out_shape = tensor.shape[:-1] + (OUT,)
    x = np.ascontiguousarray(tensor.reshape(-1, IN))
    sel = np.asarray(bank_selections).reshape(-1).astype(np.int64)
    weight = np.asarray(weight, dtype=np.float32)
    bias = np.asarray(bias, dtype=np.float32)
    n_tok = sel.shape[0]

    order = np.argsort(sel, kind="stable")
    counts = np.bincount(sel, minlength=NB)
    starts = np.concatenate(([0], np.cumsum(counts)))

    # jobs: (bank, token index array), each <= 128 tokens
    jobs = []
    for e in range(NB):
        idx = order[starts[e]:starts[e + 1]]
        if len(idx) <= P:
            jobs.append((e, idx))
        else:
            for lo in range(0, len(idx), P):
                jobs.append((e, idx[lo:lo + P]))
    # pad job count to a multiple of N_CORES
    while len(jobs) % N_CORES:
        jobs.append((0, np.empty(0, np.int64)))
    jpc = len(jobs) // N_CORES
    cap = max(16, -(-max(len(idx) for _, idx in jobs) // 16) * 16)

    kt = IN // P
    XT = np.zeros((N_CORES, jpc, kt, P, cap), np.float32)
    WT = np.empty((N_CORES, jpc, kt, P, OUT), np.float32)
    for j, (e, idx) in enumerate(jobs):
        c, s = j % N_CORES, j // N_CORES
        if len(idx):
            XT[c, s].reshape(IN, cap)[:, :len(idx)] = x[idx].T
        WT[c, s] = weight[e].T.reshape(kt, P, OUT)
    # device wants x as [p, j, k, t] and w as [j, p, k, n], both contiguous
    np_dt = mybir.dt.np(MM_DT)
    XT = np.ascontiguousarray(XT.transpose(0, 3, 1, 2, 4), dtype=np_dt)
    WT = np.ascontiguousarray(WT.transpose(0, 1, 3, 2, 4), dtype=np_dt)

    nc = _get_module(jpc, cap)
    in_maps = [{"xt": XT[c], "wt": WT[c]} for c in range(N_CORES)]
    res = run_bass_kernel_spmd(nc, in_maps, core_ids=list(range(N_CORES)))
    LAST_RESULTS = res

    out_full = np.empty((n_tok, OUT), np.float32)
    for j, (e, idx) in enumerate(jobs):
        if not len(idx):
            continue
        c, s = j % N_CORES, j // N_CORES
        out_full[idx] = res.results[c]["out"][s, :len(idx)] + bias[e]
    return out_full.reshape(out_shape)
